# revision 2
# baseline (speedup 1.0000x reference)
"""Node2Vec loss kernel for Trainium2 (8 NeuronCores, Bass/Tile).

Strategy (data parallel): replicate the embedding table X on all 8 cores,
split the 4096 random-walk rows 512 per core. Each core:
  - gathers its 512*16 = 8192 embedding rows from HBM via 64 indirect
    DMAs (one offset per partition, 128 rows each, 4 SWDGE queues),
  - computes dots[b,j] = <emb[b,j], emb[b,0]> on the vector engine with
    fused multiply+reduce (batch rows on partitions),
  - applies the host-precomputed dedup mask additively pre-exp,
  - reduces to the per-row loss l*ln(denom) - numerator.
Host averages the 4096 per-row losses (no collectives needed).
"""
import sys

sys.path.insert(0, "/opt/trn_rl_repo")
import numpy as np

from concourse import bass, bacc, mybir
import concourse.tile as tile
from concourse.bass_utils import run_bass_kernel_spmd

N_NODES = 500000
DIM = 128
BATCH = 4096
ROW = 16
P = 128
NCORES = 8
BPC = BATCH // NCORES      # 512 batch rows per core
T = BPC // P               # 4 partition-tiles of batch rows
G = T * ROW                # 64 gather slots per partition
NEG_BIG = -30.0            # additive mask: exp(-30) ~ 1e-13, below f32 eps of denom


def _build_program(l):
    W = l + 1
    nc = bacc.Bacc("TRN2", target_bir_lowering=False, debug=True,
                   num_swdge_queues=4)
    X_p = nc.declare_dram_parameter("X", [N_NODES, DIM], mybir.dt.float32,
                                    isOutput=False)
    idx_p = nc.declare_dram_parameter("idx", [P, G], mybir.dt.int32,
                                      isOutput=False)
    am_p = nc.declare_dram_parameter("amask", [P, G], mybir.dt.float32,
                                     isOutput=False)
    out_p = nc.declare_dram_parameter("loss", [P, T], mybir.dt.float32,
                                      isOutput=True)

    with tile.TileContext(nc) as tc:
        with tc.tile_pool(name="p", bufs=1) as pool:
            idx_t = pool.tile([P, G], mybir.dt.int32)
            am_t = pool.tile([P, G], mybir.dt.float32)
            embs = [
                pool.tile([P, DIM], mybir.dt.float32, name=f"e{g}", tag=f"e{g}")
                for g in range(G)
            ]
            scratch = pool.tile([P, DIM], mybir.dt.float32)
            dots_t = pool.tile([P, G], mybir.dt.float32)
            dm_t = pool.tile([P, G], mybir.dt.float32)
            ex_t = pool.tile([P, G], mybir.dt.float32)
            numer_t = pool.tile([P, T], mybir.dt.float32)
            denom_t = pool.tile([P, T], mybir.dt.float32)
            lnd_t = pool.tile([P, T], mybir.dt.float32)
            loss_t = pool.tile([P, T], mybir.dt.float32)

            nc.sync.dma_start(out=idx_t[:], in_=idx_p[:])
            nc.sync.dma_start(out=am_t[:], in_=am_p[:])

            for g in range(G):
                nc.gpsimd.indirect_dma_start(
                    out=embs[g][:],
                    out_offset=None,
                    in_=X_p[:],
                    in_offset=bass.IndirectOffsetOnAxis(
                        ap=idx_t[:, g : g + 1], axis=0
                    ),
                )

            # dots[p, g] = <embs[g][p], embs[16*t][p]>  (start row of tile t)
            for t in range(T):
                start = embs[t * ROW]
                for j in range(ROW):
                    g = t * ROW + j
                    nc.vector.tensor_tensor(
                        out=scratch[:],
                        in0=embs[g][:],
                        in1=start[:],
                        op=mybir.AluOpType.mult,
                    )
                    nc.vector.tensor_reduce(
                        out=dots_t[:, g : g + 1],
                        in_=scratch[:],
                        axis=mybir.AxisListType.X,
                        op=mybir.AluOpType.add,
                    )

            # numerator[p, t] = sum_j=1..l dots[p, t*ROW + j]
            nc.vector.tensor_reduce(
                out=numer_t[:],
                in_=dots_t[:].rearrange("p (t j) -> p t j", t=T)[:, :, 1:W],
                axis=mybir.AxisListType.X,
                op=mybir.AluOpType.add,
            )

            # masked exp: ex = exp(dots + amask)
            nc.vector.tensor_tensor(
                out=dm_t[:], in0=dots_t[:], in1=am_t[:], op=mybir.AluOpType.add
            )
            nc.scalar.activation(
                out=ex_t[:], in_=dm_t[:], func=mybir.ActivationFunctionType.Exp
            )

            # denom[p, t] = sum_j ex[p, t*ROW + j]
            nc.vector.tensor_reduce(
                out=denom_t[:],
                in_=ex_t[:].rearrange("p (t j) -> p t j", t=T),
                axis=mybir.AxisListType.X,
                op=mybir.AluOpType.add,
            )

            # loss = l * ln(denom) - numerator
            nc.scalar.activation(
                out=lnd_t[:], in_=denom_t[:], func=mybir.ActivationFunctionType.Ln
            )
            nc.vector.tensor_scalar(
                out=lnd_t[:], in0=lnd_t[:], scalar1=float(l), scalar2=None,
                op0=mybir.AluOpType.mult,
            )
            nc.vector.tensor_tensor(
                out=loss_t[:], in0=lnd_t[:], in1=numer_t[:],
                op=mybir.AluOpType.subtract,
            )
            nc.sync.dma_start(out=out_p[:], in_=loss_t[:])

    nc.compile()
    return nc


def _host_preprocess(rw_batch, l):
    """Per-core idx [P, G] int32 and additive mask [P, G] f32."""
    W = l + 1
    rw = np.asarray(rw_batch)
    idxs, masks = [], []
    for c in range(NCORES):
        rows = rw[c * BPC : (c + 1) * BPC]  # [BPC, ROW]
        idx = np.empty((P, G), np.int32)
        am = np.zeros((P, G), np.float32)
        for t in range(T):
            blk = rows[t * P : (t + 1) * P]  # [P, ROW]
            idx[:, t * ROW : (t + 1) * ROW] = blk.astype(np.int32)
            # dedup mask on the walk portion: drop j<W seen earlier in walk
            walk = blk[:, :W]  # [P, W]
            dup = np.zeros((P, W), bool)
            for j in range(1, W):
                dup[:, j] = (walk[:, :j] == walk[:, j : j + 1]).any(axis=1)
            am[:, t * ROW : t * ROW + W][dup] = NEG_BIG
        idxs.append(idx)
        masks.append(am)
    return idxs, masks


_prog_cache = {}


def kernel(X, rw_batch, l):
    l = int(l)
    X = np.ascontiguousarray(np.asarray(X, dtype=np.float32))
    assert X.shape == (N_NODES, DIM)
    assert np.asarray(rw_batch).shape == (BATCH, ROW)

    if l not in _prog_cache:
        _prog_cache[l] = _build_program(l)
    nc = _prog_cache[l]

    idxs, masks = _host_preprocess(rw_batch, l)
    in_maps = [
        {"X": X, "idx": idxs[c], "amask": masks[c]} for c in range(NCORES)
    ]
    res = run_bass_kernel_spmd(nc, in_maps, list(range(NCORES)))
    total = 0.0
    for c in range(NCORES):
        total += float(np.asarray(res.results[c]["loss"], dtype=np.float64).sum())
    return np.float32(total / BATCH)


# revision 3
# speedup vs baseline: 1.1250x; 1.1250x over previous
"""Node2Vec loss kernel for Trainium2 (8 NeuronCores, Bass/Tile).

Strategy (data parallel): replicate the embedding table X on all 8 cores,
split the 4096 random-walk rows 512 per core. Each core:
  - gathers its 512*16 = 8192 embedding rows from HBM via 64 indirect
    DMAs (one offset per partition, 128 rows each, 4 SWDGE queues),
  - computes dots[b,j] = <emb[b,j], emb[b,0]> on the vector engine with
    fused multiply+reduce (batch rows on partitions),
  - applies the host-precomputed dedup mask additively pre-exp,
  - reduces to the per-row loss l*ln(denom) - numerator.
Host averages the 4096 per-row losses (no collectives needed).
"""
import sys

sys.path.insert(0, "/opt/trn_rl_repo")
import numpy as np

from concourse import bass, bacc, mybir
import concourse.tile as tile
from concourse.bass_utils import run_bass_kernel_spmd

N_NODES = 500000
DIM = 128
BATCH = 4096
ROW = 16
P = 128
NCORES = 8
BPC = BATCH // NCORES      # 512 batch rows per core
T = BPC // P               # 4 partition-tiles of batch rows
G = T * ROW                # 64 gather slots per partition
NEG_BIG = -30.0            # additive mask: exp(-30) ~ 1e-13, below f32 eps of denom


def _build_program(l):
    W = l + 1
    nc = bacc.Bacc("TRN2", target_bir_lowering=False, debug=True,
                   num_swdge_queues=4)
    X_p = nc.declare_dram_parameter("X", [N_NODES, DIM], mybir.dt.float32,
                                    isOutput=False)
    idx_p = nc.declare_dram_parameter("idx", [P, G], mybir.dt.int32,
                                      isOutput=False)
    am_p = nc.declare_dram_parameter("amask", [P, G], mybir.dt.float32,
                                     isOutput=False)
    out_p = nc.declare_dram_parameter("loss", [P, T], mybir.dt.float32,
                                      isOutput=True)

    with tile.TileContext(nc) as tc:
        with tc.tile_pool(name="p", bufs=1) as pool:
            idx_t = pool.tile([P, G], mybir.dt.int32)
            am_t = pool.tile([P, G], mybir.dt.float32)
            embs = [
                pool.tile([P, DIM], mybir.dt.float32, name=f"e{g}", tag=f"e{g}")
                for g in range(G)
            ]
            scratch = pool.tile([P, DIM], mybir.dt.float32)
            dots_t = pool.tile([P, G], mybir.dt.float32)
            dm_t = pool.tile([P, G], mybir.dt.float32)
            ex_t = pool.tile([P, G], mybir.dt.float32)
            numer_t = pool.tile([P, T], mybir.dt.float32)
            denom_t = pool.tile([P, T], mybir.dt.float32)
            lnd_t = pool.tile([P, T], mybir.dt.float32)
            loss_t = pool.tile([P, T], mybir.dt.float32)

            nc.sync.dma_start(out=idx_t[:], in_=idx_p[:])
            nc.sync.dma_start(out=am_t[:], in_=am_p[:])

            for g in range(G):
                nc.gpsimd.indirect_dma_start(
                    out=embs[g][:],
                    out_offset=None,
                    in_=X_p[:],
                    in_offset=bass.IndirectOffsetOnAxis(
                        ap=idx_t[:, g : g + 1], axis=0
                    ),
                )

            # dots[p, g] = <embs[g][p], embs[16*t][p]>  (start row of tile t)
            for t in range(T):
                start = embs[t * ROW]
                for j in range(ROW):
                    g = t * ROW + j
                    nc.vector.tensor_tensor(
                        out=scratch[:],
                        in0=embs[g][:],
                        in1=start[:],
                        op=mybir.AluOpType.mult,
                    )
                    nc.vector.tensor_reduce(
                        out=dots_t[:, g : g + 1],
                        in_=scratch[:],
                        axis=mybir.AxisListType.X,
                        op=mybir.AluOpType.add,
                    )

            # numerator[p, t] = sum_j=1..l dots[p, t*ROW + j]
            nc.vector.tensor_reduce(
                out=numer_t[:],
                in_=dots_t[:].rearrange("p (t j) -> p t j", t=T)[:, :, 1:W],
                axis=mybir.AxisListType.X,
                op=mybir.AluOpType.add,
            )

            # masked exp: ex = exp(dots + amask)
            nc.vector.tensor_tensor(
                out=dm_t[:], in0=dots_t[:], in1=am_t[:], op=mybir.AluOpType.add
            )
            nc.scalar.activation(
                out=ex_t[:], in_=dm_t[:], func=mybir.ActivationFunctionType.Exp
            )

            # denom[p, t] = sum_j ex[p, t*ROW + j]
            nc.vector.tensor_reduce(
                out=denom_t[:],
                in_=ex_t[:].rearrange("p (t j) -> p t j", t=T),
                axis=mybir.AxisListType.X,
                op=mybir.AluOpType.add,
            )

            # loss = l * ln(denom) - numerator
            nc.scalar.activation(
                out=lnd_t[:], in_=denom_t[:], func=mybir.ActivationFunctionType.Ln
            )
            nc.vector.tensor_scalar(
                out=lnd_t[:], in0=lnd_t[:], scalar1=float(l), scalar2=None,
                op0=mybir.AluOpType.mult,
            )
            nc.vector.tensor_tensor(
                out=loss_t[:], in0=lnd_t[:], in1=numer_t[:],
                op=mybir.AluOpType.subtract,
            )
            nc.sync.dma_start(out=out_p[:], in_=loss_t[:])

    nc.compile()
    return nc


def _host_preprocess(rw_batch, l):
    """Per-core idx [P, G] int32 and additive mask [P, G] f32."""
    W = l + 1
    rw = np.asarray(rw_batch)
    idxs, masks = [], []
    for c in range(NCORES):
        rows = rw[c * BPC : (c + 1) * BPC]  # [BPC, ROW]
        idx = np.empty((P, G), np.int32)
        am = np.zeros((P, G), np.float32)
        for t in range(T):
            blk = rows[t * P : (t + 1) * P]  # [P, ROW]
            idx[:, t * ROW : (t + 1) * ROW] = blk.astype(np.int32)
            # dedup mask on the walk portion: drop j<W seen earlier in walk
            walk = blk[:, :W]  # [P, W]
            dup = np.zeros((P, W), bool)
            for j in range(1, W):
                dup[:, j] = (walk[:, :j] == walk[:, j : j + 1]).any(axis=1)
            am[:, t * ROW : t * ROW + W][dup] = NEG_BIG
        idxs.append(idx)
        masks.append(am)
    return idxs, masks


_prog_cache = {}


def kernel(X, rw_batch, l):
    l = int(l)
    X = np.ascontiguousarray(np.asarray(X, dtype=np.float32))
    assert X.shape == (N_NODES, DIM)
    assert np.asarray(rw_batch).shape == (BATCH, ROW)

    if l not in _prog_cache:
        _prog_cache[l] = _build_program(l)
    nc = _prog_cache[l]

    idxs, masks = _host_preprocess(rw_batch, l)
    in_maps = [
        {"X": X, "idx": idxs[c], "amask": masks[c]} for c in range(NCORES)
    ]
    res = run_bass_kernel_spmd(nc, in_maps, list(range(NCORES)))
    total = 0.0
    for c in range(NCORES):
        total += float(np.asarray(res.results[c]["loss"], dtype=np.float64).sum())
    return np.asarray(total / BATCH, dtype=np.float32)
